# revision 3
# baseline (speedup 1.0000x reference)
"""Trainium2 Bass kernel for an autoregressive LSTMCell decoder with softmax feedback.

Math (per timestep, PyTorch gate order i,f,g,o):
    gates = [x_t, y] @ W_ih.T + b_ih + h @ W_hh.T + b_hh
    i,f,o = sigmoid(...), g = tanh(...)
    c = f*c + i*g ; h = o*tanh(c) ; y = softmax(h @ W_lin.T + b_lin)

Strategy (8 NeuronCores, data parallel over batch, 32 sequences/core):
  * Features-on-partitions: gates.T [2048, 32] in one PSUM bank [128, 512]
    (free = 16 m-blocks x 32 batch); state S=2c / H=2h packed [128, (k,b)].
  * x @ W_x.T prefilled DIRECTLY into the step's PSUM bank 3 steps ahead
    (64 fp16 matmuls/step as PE filler during the recurrent tail).
  * Recurrent + feedback matmuls in fp8e4 DoubleRow (2 k-chunks per
    instruction, 0.5 cy/row): W_hh = 32 instrs, W_y+bias folded into 16
    "pair C" instrs (A0=W_y padded, A1=bias hi/lo rows vs ones columns),
    logits = 2 DR instrs + 1 bias matmul.
  * All gate preacts scaled x4 in PSUM so fp8 weights stay in e4m3 normal
    range; compensated for free via ACT scale=0.25 on the tanh/exp reads.
  * Tail: tanh on ACT; the 3-op cell update (u,v,S') and the H=2h fp8
    write on the Pool (gpsimd) engine - 107ns/op, no access-latency
    charge, and it leaves DVE free for the softmax/y path.
  * sigmoid via tanh identity (weights pre-scaled 0.5 host-side):
    sigma(x) = (1 + tanh(x/2))/2; only the Exp/Tanh ACT table is used.
"""

import sys

sys.path.insert(0, "/opt/trn_rl_repo")

import numpy as np
import ml_dtypes

import concourse.bass as bass  # noqa: F401
import concourse.tile as tile
from concourse import bacc, mybir
from concourse.bass_utils import run_bass_kernel_spmd

f32 = mybir.dt.float32
f16 = mybir.dt.float16
f8 = mybir.dt.float8e4
AF = mybir.ActivationFunctionType
ALU = mybir.AluOpType
DR = mybir.MatmulPerfMode.DoubleRow
F8 = ml_dtypes.float8_e4m3

B, D, N = 256, 512, 64
NCORES = 8
BL = B // NCORES  # 32 sequences per core
TC = 32           # timesteps per x-staging chunk
P = 128
AHEAD = 3         # prefill distance (PSUM banks used = AHEAD + 1)
GS = 4.0          # gate-preact scale (fp8 normal range); ACT scale=1/GS

_CACHE = {}
import os
SKIP_DR = os.environ.get("SKIP_DR", "0") == "1"
SKIP_WHH = os.environ.get("SKIP_WHH", "0") == "1"
DBG_DUMP = os.environ.get("DBG_DUMP", "0") == "1"


def _build(T, reps=1):
    NCH = T // TC
    nc = bacc.Bacc("TRN2", target_bir_lowering=False, debug=os.environ.get("BASS_DEBUG","0")=="1", num_devices=NCORES)

    x_d = nc.dram_tensor("x", [NCH, P, 4 * TC * BL], f16, kind="ExternalInput").ap()
    h0_d = nc.dram_tensor("h0", [P, 4 * BL], f8, kind="ExternalInput").ap()
    wx_d = nc.dram_tensor("wx", [P, 64 * P], f16, kind="ExternalInput").ap()
    wh_d = nc.dram_tensor("wh", [P, 64 * P], f8, kind="ExternalInput").ap()
    wyc_d = nc.dram_tensor("wyc", [P, 32 * P], f8, kind="ExternalInput").ap()
    wl_d = nc.dram_tensor("wl", [P, 256], f16, kind="ExternalInput").ap()
    bl_d = nc.dram_tensor("bl", [1, N], f16, kind="ExternalInput").ap()
    out_d = nc.dram_tensor("out", [BL, T, N], f32, kind="ExternalOutput").ap()
    if DBG_DUMP:
        dmp = {nm: nc.dram_tensor(f"dbg_{nm}", shp, dt, kind="ExternalOutput").ap()
               for nm, shp, dt in [
                   ("g", [P, 512], f32), ("Tg", [P, 384], f16),
                   ("Tgo", [P, 128], f16), ("S", [P, 128], f32),
                   ("Tc", [P, 128], f16), ("H16", [P, 128], f16),
                   ("l", [BL, N], f32), ("e", [BL, N], f16),
                   ("z", [BL, 1], f32), ("y16", [BL, N], f16),
                   ("ypk", [P, 2 * BL], f8)]}

    with tile.TileContext(nc) as tc:
        with (
            tc.tile_pool(name="const", bufs=1) as const,
            tc.tile_pool(name="xst", bufs=2) as xst,
            tc.tile_pool(name="yout", bufs=2) as yout,
            tc.tile_pool(name="state", bufs=2) as state,
            tc.tile_pool(name="tmp", bufs=3) as tmp,
            tc.tile_pool(name="g_ps", bufs=AHEAD + 1, space="PSUM") as g_psp,
            tc.tile_pool(name="l_ps", bufs=2, space="PSUM") as l_psp,
        ):
            # ---- constants / weights ----
            wx = const.tile([P, 64 * P], f16)
            nc.sync.dma_start(out=wx, in_=wx_d)
            wh = const.tile([P, 64 * P], f8)
            nc.sync.dma_start(out=wh, in_=wh_d)
            wyc = const.tile([P, 32 * P], f8)
            nc.sync.dma_start(out=wyc, in_=wyc_d)
            wl = const.tile([P, 256], f16)
            nc.sync.dma_start(out=wl, in_=wl_d)
            bl = const.tile([1, N], f16)
            nc.sync.dma_start(out=bl, in_=bl_d)
            ones1 = const.tile([1, BL], f16)
            nc.vector.memset(ones1, 1.0)

            for _rep in range(reps):
                # ---- initial state ----
                H8 = state.tile([P, 4 * BL], f8, tag="H")
                nc.sync.dma_start(out=H8, in_=h0_d)
                S = state.tile([P, 4 * BL], f32, tag="S")
                nc.vector.memset(S, 0.0)
                ypk = const.tile([P, 2 * BL], f8, name=f"ypk{_rep}")
                nc.vector.memset(ypk, 0.0)
                nc.vector.memset(ypk[0:2, BL:2 * BL], 1.0)

                def stage_x(ch):
                    xT = xst.tile([P, 4 * TC * BL], f16, tag="xT")
                    nc.sync.dma_start(out=xT, in_=x_d[ch])
                    return xT.rearrange("p (k t b) -> p k t b", k=4, t=TC)

                gq = []

                def prefill(s, xv):
                    g = g_psp.tile([P, 512], f32, tag="g")
                    tt = s % TC
                    for k in range(4):
                        for m in range(16):
                            nc.tensor.matmul(
                                g[:, 32 * m:32 * m + 32],
                                wx[:, (k * 16 + m) * P:(k * 16 + m + 1) * P],
                                xv[:, k, tt, :],
                                start=(k == 0 and m == 0), stop=False,
                                skip_group_check=True,
                            )
                    gq.append(g)

                xv_cur = stage_x(0)
                xv_next = None
                for s in range(min(AHEAD, T)):
                    prefill(s, xv_cur)

                for ch in range(NCH):
                    if ch + 1 < NCH:
                        xv_next = stage_x(ch + 1)
                    Y = yout.tile([BL, TC * N], f32, tag="Y")

                    for tt in range(TC):
                        t = ch * TC + tt
                        g = gq.pop(0)
                        # --- PE: W_hh in fp8 DoubleRow (2 k-chunks/instr) ---
                        for m in range(0 if (SKIP_DR or SKIP_WHH) else 16):
                            for p2 in range(2):
                                nc.tensor.matmul(
                                    g[:, 32 * m:32 * m + 32],
                                    wh[:, (p2 * 16 + m) * 2 * P:
                                        ((p2 * 16 + m) * 2 + 2) * P].rearrange(
                                        "r (two q) -> r two q", two=2),
                                    H8[:, 64 * p2:64 * p2 + 64].rearrange(
                                        "r (two b) -> r two b", two=2),
                                    start=False, stop=False, perf_mode=DR,
                                    skip_group_check=True,
                                )
                        # --- PE: pair C = W_y feedback + gate bias ---
                        for m in range(15 if SKIP_DR else 0, 16):
                            nc.tensor.matmul(
                                g[:, 32 * m:32 * m + 32],
                                wyc[:, 2 * m * P:(2 * m + 2) * P].rearrange(
                                    "r (two q) -> r two q", two=2),
                                ypk.rearrange("r (two b) -> r two b", two=2),
                                start=False, stop=(m == 15), perf_mode=DR,
                                skip_group_check=True,
                            )
                        # --- PE filler: prefill step t+AHEAD ---
                        s2 = t + AHEAD
                        if s2 < T:
                            prefill(s2, xv_cur if s2 // TC == ch else xv_next)

                        # --- PE: logits bias early (no deps) ---
                        l_full = l_psp.tile([BL, 512], f32, tag="l")
                        l_ps = l_full[:, 0:N]
                        nc.tensor.matmul(l_ps, ones1, bl, start=True, stop=False,
                                         skip_group_check=True)
                        # --- ACT: gate tanh (i,f,g then o) ---
                        Tg = tmp.tile([P, 384], f16, tag="Tg")
                        nc.scalar.activation(out=Tg, in_=g[:, 0:384],
                                             func=AF.Tanh, scale=1.0 / GS)
                        Tgo = tmp.tile([P, 4 * BL], f16, tag="Tgo")
                        nc.scalar.activation(out=Tgo, in_=g[:, 384:512],
                                             func=AF.Tanh, scale=1.0 / GS)
                        # --- Pool: cell update (S=2c) ---
                        u = tmp.tile([P, 4 * BL], f32, tag="u")
                        nc.vector.scalar_tensor_tensor(
                            out=u, in0=Tg[:, 128:256], scalar=1.0, in1=S,
                            op0=ALU.add, op1=ALU.mult)
                        v = tmp.tile([P, 4 * BL], f32, tag="v")
                        nc.vector.scalar_tensor_tensor(
                            out=v, in0=Tg[:, 0:128], scalar=1.0, in1=Tg[:, 256:384],
                            op0=ALU.add, op1=ALU.mult)
                        S = state.tile([P, 4 * BL], f32, tag="S")
                        nc.vector.scalar_tensor_tensor(
                            out=S, in0=u, scalar=0.5, in1=v,
                            op0=ALU.mult, op1=ALU.add)
                        # --- ACT: tanh(c) ; Pool: H = 2h in fp8 ---
                        Tc_ = tmp.tile([P, 4 * BL], f16, tag="Tc")
                        nc.scalar.activation(out=Tc_, in_=S, func=AF.Tanh, scale=0.5)
                        H16 = state.tile([P, 4 * BL], f16, tag="H16")
                        nc.vector.scalar_tensor_tensor(
                            out=H16, in0=Tgo, scalar=1.0, in1=Tc_,
                            op0=ALU.add, op1=ALU.mult)
                        H8 = state.tile([P, 4 * BL], f8, tag="H")
                        nc.vector.scalar_tensor_tensor(
                            out=H8, in0=Tgo, scalar=1.0, in1=Tc_,
                            op0=ALU.add, op1=ALU.mult)
                        # --- PE: logits from the NEW h (fp16) ---
                        for k in range(4):
                            nc.tensor.matmul(
                                l_ps,
                                H16[:, 32 * k:32 * k + 32],
                                wl[:, 64 * k:64 * k + 64],
                                start=False, stop=(k == 3),
                                skip_group_check=True,
                            )
                        if DBG_DUMP and t == 0:
                            gsb = tmp.tile([P, 512], f32, name="gsb")
                            nc.vector.tensor_copy(out=gsb, in_=g)
                            nc.sync.dma_start(out=dmp["g"], in_=gsb)
                            nc.sync.dma_start(out=dmp["Tg"], in_=Tg)
                            nc.sync.dma_start(out=dmp["Tgo"], in_=Tgo)
                            nc.sync.dma_start(out=dmp["S"], in_=S)
                            nc.sync.dma_start(out=dmp["Tc"], in_=Tc_)
                            nc.sync.dma_start(out=dmp["H16"], in_=H16)
                            lsb = tmp.tile([BL, N], f32, name="lsb")
                            nc.vector.tensor_copy(out=lsb, in_=l_ps)
                            nc.sync.dma_start(out=dmp["l"], in_=lsb)
                            nc.sync.dma_start(out=dmp["ypk"], in_=ypk)
                        # --- ACT+DVE: softmax y, feedback transpose, output ---
                        e = tmp.tile([BL, N], f16, tag="e")
                        nc.scalar.activation(out=e, in_=l_ps, func=AF.Exp,
                                             scale=1.0 / GS)
                        z = tmp.tile([BL, 1], f32, tag="z")
                        nc.vector.tensor_reduce(out=z, in_=e,
                                                axis=mybir.AxisListType.X,
                                                op=ALU.add)
                        rz = tmp.tile([BL, 1], f32, tag="rz")
                        nc.vector.reciprocal(rz, z)
                        y16 = tmp.tile([BL, N], f16, tag="y16")
                        nc.vector.tensor_scalar(out=y16, in0=e, scalar1=rz,
                                                scalar2=None, op0=ALU.mult)
                        yT = tmp.tile([N, BL], f16, tag="yT")
                        nc.vector.transpose(out=yT[0:32, :], in_=y16[:, 0:32])
                        nc.vector.transpose(out=yT[32:64, :], in_=y16[:, 32:64])
                        nc.vector.tensor_copy(out=ypk[0:N, 0:BL], in_=yT)
                        nc.vector.tensor_scalar(out=Y[:, tt * N:(tt + 1) * N],
                                                in0=e, scalar1=rz, scalar2=None,
                                                op0=ALU.mult)
                        if DBG_DUMP and t == 0:
                            nc.sync.dma_start(out=dmp["e"], in_=e)
                            nc.sync.dma_start(out=dmp["z"], in_=z)
                            nc.sync.dma_start(out=dmp["y16"], in_=y16)

                    nc.sync.dma_start(
                        out=out_d[:, ch * TC:(ch + 1) * TC, :].rearrange(
                            "b t n -> b (t n)"),
                        in_=Y,
                    )
                    xv_cur = xv_next

    nc.compile()
    return nc


def _f8(a):
    return np.asarray(a, np.float32).astype(F8)


def _prep(W_ih, b_ih, W_hh, b_hh, W_lin, b_lin):
    sg = np.concatenate([
        np.full(D, 0.5), np.full(D, 0.5), np.ones(D), np.full(D, 0.5)
    ]).astype(np.float32)

    # prefill weights: x4 gate scale, fp16
    W_x = (W_ih[:, :D] * sg[:, None] * GS).astype(np.float16)
    wx = W_x.reshape(16, P, 4, P).transpose(3, 2, 0, 1).reshape(P, 64 * P).copy()

    # W_hh fp8 DoubleRow packing: [r, (p, m, i, q)]
    W_h2 = (W_hh * sg[:, None] * 0.5 * GS).astype(np.float32)
    wh = (W_h2.reshape(16, P, 4, P).transpose(3, 2, 0, 1)  # [r, k, m, q]
          .reshape(P, 2, 2, 16, P).transpose(0, 1, 3, 2, 4)  # [r, p, m, i, q]
          .reshape(P, 64 * P))
    wh8 = _f8(wh)

    # pair C: A0 = W_y padded, A1 = bias hi/lo rows; [r, (m, i, q)]
    W_y2 = (W_ih[:, D:] * sg[:, None] * GS).astype(np.float32)  # [2048, 64]
    bias = ((b_ih + b_hh) * sg * GS).astype(np.float32)
    b_hi = bias.astype(F8).astype(np.float32)
    b_lo = bias - b_hi
    wyc = np.zeros((P, 16, 2, P), np.float32)
    wyc[0:64, :, 0, :] = W_y2.reshape(16, P, 64).transpose(2, 0, 1)
    wyc[0, :, 1, :] = b_hi.reshape(16, P)
    wyc[1, :, 1, :] = b_lo.reshape(16, P)
    wyc8 = _f8(wyc.reshape(P, 32 * P))

    # logits: [r, (p, i, n)] fp8, x4 scale; H=2h -> x0.5
    W_l2 = (W_lin * 0.5 * GS).astype(np.float32)
    wl16 = W_l2.reshape(N, 4, P).transpose(2, 1, 0).reshape(P, 256).astype(
        np.float16).copy()

    return dict(
        wx=wx, wh=wh8, wyc=wyc8, wl=wl16,
        bl=(b_lin * GS).astype(np.float16).reshape(1, N).copy(),
    )


def make_in_maps(x, init_h, W_ih, b_ih, W_hh, b_hh, W_lin, b_lin):
    x = np.asarray(x, dtype=np.float32)
    T = x.shape[1]
    assert x.shape == (B, T, D) and T % TC == 0
    shared = _prep(np.asarray(W_ih, np.float32), np.asarray(b_ih, np.float32),
                   np.asarray(W_hh, np.float32), np.asarray(b_hh, np.float32),
                   np.asarray(W_lin, np.float32), np.asarray(b_lin, np.float32))
    init_h = np.asarray(init_h, np.float32)

    in_maps = []
    for i in range(NCORES):
        m = dict(shared)
        xc = x[i * BL:(i + 1) * BL]  # [BL, T, D]
        xc = xc.reshape(BL, T // TC, TC, 4, P).transpose(1, 4, 3, 2, 0)
        m["x"] = np.ascontiguousarray(xc).reshape(T // TC, P, 4 * TC * BL).astype(
            np.float16)
        h0 = init_h[i * BL:(i + 1) * BL]  # [BL, D]
        m["h0"] = _f8(2.0 * h0.reshape(BL, 4, P).transpose(2, 1, 0).reshape(
            P, 4 * BL))
        in_maps.append(m)
    return in_maps, T


def kernel(x, init_h, W_ih, b_ih, W_hh, b_hh, W_lin, b_lin, _trace=False):
    in_maps, T = make_in_maps(x, init_h, W_ih, b_ih, W_hh, b_hh, W_lin, b_lin)
    if T not in _CACHE:
        _CACHE[T] = _build(T)
    nc = _CACHE[T]

    res = run_bass_kernel_spmd(nc, in_maps, list(range(NCORES)), trace=_trace)
    out = np.concatenate([res.results[i]["out"] for i in range(NCORES)], axis=0)
    if _trace:
        kernel.last_exec_time_ns = res.exec_time_ns
        kernel.last_results = res
    return out
